# revision 1
# baseline (speedup 1.0000x reference)
"""Trainium2 Bass kernel for nn_GCN_5403068858882 (GCN + 3x GENConv + pool head).

Self-contained: schedule builder + bass program builder + SPMD runner.

- 8 cores, core c owns graphs [32c,32c+32) (contiguous nodes, batch sorted).
- Nodes packed into 32-slot bins by first-fit-decreasing (caps: 32 nodes,
  3*128 "A" edges, 3*128 "B" edges; A = src graph < G/2 so int16 gather
  indices fit each table half). Slots 0/1 reserved as pooling pad tokens.
- All four edge phases (conv + 3 GEN layers) ride one bf16 2H-channel
  machinery: node phase writes table rows [exp(t*v), v*exp(t*v)] (conv:
  [h0*dinv | 0]) -> AllGather -> per 128-edge tile: dma_gather rows (1024
  idx/call max - bigger calls crash the SWDGE ucode) + PE matmul with an
  is_equal selection accumulating per-bin sums in PSUM -> drain.
- The NEXT layer's node phase (LN via bn_stats + exp(-0.5*ln(var+eps)),
  PReLU, exp) is interleaved into the drain stream per 4-block group, and
  table rows stream to DRAM per group, so DMA stays saturated across phase
  boundaries. One activation-table set (ln/exp/relu) is loaded once.
- Ledger (residual history) is bf16 and doubles as the pooling gather
  source; mean/max pooling runs per 2-graph chunk (transpose SBUF gather +
  reduce), then a tiny AllGather + MLP head.
- Pooled features + readout-MLP weights are bf16 (rel err 1.06e-2 vs the
  2e-2 gate; was 6.0e-3 with an f32 head).
- TimelineSim (collectives mocked): ~1.156 ms; memory-bound on the edge
  gathers' per-descriptor cost (256B rows, 2x sub-512B latency penalty),
  with Pool-engine SWDGE prep just below the DMA roof.
"""

import numpy as np
import ml_dtypes

import concourse.bass as bass
import concourse.bacc as bacc
import concourse.mybir as mybir
import concourse.tile as tile
from concourse.bass_utils import run_bass_kernel_spmd
from concourse._compat import get_trn_type

F32 = mybir.dt.float32
BF16 = mybir.dt.bfloat16
I16 = mybir.dt.int16
AF = mybir.ActivationFunctionType
ALU = mybir.AluOpType
NPBF = ml_dtypes.bfloat16

H = 64
F_IN = 5
L = 3
EPS_BN = 1e-5
EPS_MSG = 1e-7
NCORES = 8
TA = 3
TB = 3
BINCAP = 32
CHUNK_BINS = 16          # bins per gather superchunk
MOCK_COLLECTIVES = False  # replace AllGathers with local DMA (TimelineSim)
PHASES = 3               # debug: 1=conv only, 2=+GEN layers, 3=+pool/head
CONV_AG = True           # debug: run the conv AllGather
CONV_EDGE = True         # debug: run the conv edge phase
EDGE_GATHER = True       # debug: issue dma_gather calls
EDGE_MM = True           # debug: issue edge matmuls
GATHER_SPLIT = 6         # sub-calls per gather (ring-capacity control)
DMA_SCRATCH = 32768      # SWDGE ring bytes/partition (ring = /16 descriptors, pow2)
NQUEUES = 2


# ---------------------------------------------------------------- schedule
class Sched:
    pass


def build_schedule(edge_index, batch_idx, G):
    s = Sched()
    src = np.asarray(edge_index[0], np.int64)
    dst = np.asarray(edge_index[1], np.int64)
    batch = np.asarray(batch_idx, np.int64)
    n = batch.shape[0]
    s.G = G
    s.GPC = GPC = G // NCORES

    deg = np.bincount(dst, minlength=n).astype(np.float64) + 1.0
    s.dinv_node = (deg ** -0.5).astype(np.float32)

    a_edge = batch[src] < (G // 2)
    acnt = np.bincount(dst[a_edge], minlength=n)
    bcnt = np.bincount(dst[~a_edge], minlength=n)

    gstart = np.searchsorted(batch, np.arange(G))
    gend = np.searchsorted(batch, np.arange(G), side="right")
    s.cnt = cnt = gend - gstart

    CAP_A, CAP_B = TA * 128, TB * 128
    core_bins = []
    for c in range(NCORES):
        lo, hi = gstart[c * GPC], gend[(c + 1) * GPC - 1]
        # first-fit decreasing on the binding constraint (a, b, node count)
        nodes = np.arange(lo, hi)
        order = nodes[np.argsort(-(acnt[nodes] + bcnt[nodes]), kind="stable")]
        # slots 0 and 1 stay invalid (-1): they are the pooling gather's
        # max/mean pad tokens (slot 0 is memset to -inf, slot 1 stays 0)
        bins_n, bins_a, bins_b = [[-1, -1]], [0], [0]
        for nd in order:
            a, b = int(acnt[nd]), int(bcnt[nd])
            placed = False
            for i in range(len(bins_n)):
                if (
                    len(bins_n[i]) < BINCAP
                    and bins_a[i] + a <= CAP_A
                    and bins_b[i] + b <= CAP_B
                ):
                    bins_n[i].append(nd)
                    bins_a[i] += a
                    bins_b[i] += b
                    placed = True
                    break
            if not placed:
                bins_n.append([nd])
                bins_a.append(a)
                bins_b.append(b)
        core_bins.append(bins_n)

    NB = max(len(b) for b in core_bins)
    NB = -(-NB // CHUNK_BINS) * CHUNK_BINS
    s.NB = NB
    s.NSLOT = NSLOT = NB * BINCAP
    s.NBLK = NB // 4
    assert 4 * NSLOT <= 32768, NSLOT

    slot2node = np.full((NCORES, NSLOT), -1, np.int64)
    pos_of_node = np.full(n, -1, np.int64)
    for c in range(NCORES):
        for bi, bn in enumerate(core_bins[c]):
            for j, nd in enumerate(bn):
                if nd >= 0:
                    slot2node[c, bi * BINCAP + j] = nd
                    pos_of_node[nd] = c * NSLOT + bi * BINCAP + j
    assert (pos_of_node >= 0).all()
    s.slot2node, s.pos_of_node = slot2node, pos_of_node
    s.SPLIT = 4 * NSLOT

    dst_pos = pos_of_node[dst]
    dst_core = dst_pos // NSLOT
    dst_bin = (dst_pos % NSLOT) // BINCAP
    dst_slot = (dst_pos % NSLOT) % BINCAP
    # table rows are partition-major: node at local slot s (= blk*128 + p)
    # lives in DRAM row core*NSLOT + p*NBLK + blk, so the SBUF->DRAM table
    # write is contiguous per partition.
    sl = pos_of_node[src] % NSLOT
    src_pos = (pos_of_node[src] // NSLOT) * NSLOT + (sl % 128) * s.NBLK + sl // 128

    NT_A, NT_B = NB * TA, NB * TB
    idxA = np.zeros((NCORES, NT_A * 128), np.int16)
    dstA = np.full((NCORES, NT_A * 128), -1.0, np.float32)
    idxB = np.zeros((NCORES, NT_B * 128), np.int16)
    dstB = np.full((NCORES, NT_B * 128), -1.0, np.float32)

    order = np.lexsort((src_pos, dst_bin, dst_core))
    eo_src, eo_core = src_pos[order], dst_core[order]
    eo_bin, eo_slot, eo_a = dst_bin[order], dst_slot[order], a_edge[order]

    for c in range(NCORES):
        msk_c = eo_core == c
        for idxarr, dstarr, T, off, grp in (
            (idxA, dstA, TA, 0, True),
            (idxB, dstB, TB, s.SPLIT, False),
        ):
            msk = msk_c & (eo_a == grp)
            bins_e, srcs, slots = eo_bin[msk], eo_src[msk] - off, eo_slot[msk]
            bs = np.searchsorted(bins_e, np.arange(NB))
            be = np.searchsorted(bins_e, np.arange(NB), side="right")
            for bi in range(NB):
                k = be[bi] - bs[bi]
                assert k <= T * 128
                base = bi * T * 128
                idxarr[c, base : base + k] = srcs[bs[bi] : be[bi]].astype(np.int16)
                dstarr[c, base : base + k] = slots[bs[bi] : be[bi]].astype(np.float32)

    s.idxA, s.dstA, s.idxB, s.dstB = idxA, dstA, idxB, dstB

    valid = slot2node >= 0
    s.valid = valid
    s.dinv_slot = np.where(
        valid, s.dinv_node[np.clip(slot2node, 0, None)], 0.0
    ).astype(np.float32)
    s.mask_slot = valid.astype(np.float32)

    maxcnt = int(cnt.max())
    SG = max(64, -(-maxcnt // 64) * 64)   # %64 so 2-graph pool gathers are %128
    s.SG = SG
    gidx_mean = np.zeros((NCORES, GPC * SG), np.int16)
    gidx_max = np.zeros((NCORES, GPC * SG), np.int16)
    for c in range(NCORES):
        for gl in range(GPC):
            g = c * GPC + gl
            slots = (pos_of_node[np.arange(gstart[g], gend[g])] % NSLOT).astype(
                np.int16
            )
            base = gl * SG
            gidx_mean[c, base : base + len(slots)] = slots
            gidx_max[c, base : base + len(slots)] = slots
            gidx_mean[c, base + len(slots) : base + SG] = 1
            gidx_max[c, base + len(slots) : base + SG] = 0
    s.gidx_mean, s.gidx_max = gidx_mean, gidx_max
    s.inv_cnt = (1.0 / np.maximum(cnt, 1)).astype(np.float32)
    s.maxmask = (cnt > 0).astype(np.float32)
    return s


def fold_weights(w):
    f = {}
    w32 = {k: np.asarray(v, np.float32) if np.asarray(v).dtype != np.int64 else v
           for k, v in w.items()}
    sbn1 = w32["bn1_g"] / np.sqrt(1.0 + EPS_BN)
    f["Wc"] = (w32["conv1_W"] * sbn1[None, :]).astype(np.float32)
    f["btot_conv"] = (w32["conv1_b"] * sbn1 + w32["bn1_b"]).astype(np.float32)
    f["ln_g"], f["ln_b"] = w32["ln_g"], w32["ln_b"]
    f["prelu_a"], f["gen_t"] = w32["prelu_a"], w32["gen_t"]
    f["W1"], f["b1tot"], f["W2"], f["b2"] = [], [], [], []
    for i in range(L):
        smlp = w32["mlp_bn_g"][i] / np.sqrt(1.0 + EPS_BN)
        f["W1"].append((w32["mlp_W1"][i] * smlp[None, :]).astype(np.float32))
        f["b1tot"].append(
            (w32["mlp_b1"][i] * smlp + w32["mlp_bn_b"][i]).astype(np.float32)
        )
        f["W2"].append(w32["mlp_W2"][i])
        f["b2"].append(w32["mlp_b2"][i])
    for k in ("lin1_W", "lin1_b", "lin2_W", "lin2_b", "out_W", "out_b"):
        f[k] = w32[k]
    return f


def _wrap16(arr):
    """[K*16] -> [128, K] gather-idx layout (i at [i%16, i//16], tiled x8)."""
    a = np.asarray(arr, np.int16).reshape(-1, 16).T  # [16, K]
    return np.tile(a, (8, 1)).copy()


def _tile_major(arr, ntiles):
    """[ntiles*128] -> [128, ntiles] (partition = slot within tile)."""
    return np.ascontiguousarray(np.asarray(arr).reshape(ntiles, 128).T)


def build_inmaps(s, x):
    n = x.shape[0]
    NSLOT, NBLK = s.NSLOT, s.NBLK
    maps = []
    for c in range(NCORES):
        xpad = np.zeros((NSLOT, F_IN), np.float32)
        v = s.valid[c]
        xpad[v] = np.asarray(x, np.float32)[s.slot2node[c][v]]
        m = {
            "xT": np.ascontiguousarray(xpad.T).astype(NPBF),
            "idxA": _wrap16(s.idxA[c]),
            "idxB": _wrap16(s.idxB[c]),
            "dstA16": _tile_major(s.dstA[c], s.NB * TA).astype(NPBF),
            "dstB16": _tile_major(s.dstB[c], s.NB * TB).astype(NPBF),
            "dinv": np.ascontiguousarray(
                s.dinv_slot[c].reshape(NBLK, 128).T
            ),
            "mask": np.ascontiguousarray(
                s.mask_slot[c].reshape(NBLK, 128).T
            ),
            "gidxm": _wrap16(s.gidx_mean[c]),
            "gidxx": _wrap16(s.gidx_max[c]),
            "pminv": np.tile(s.inv_cnt[c * s.GPC : (c + 1) * s.GPC], (128, 1)).astype(np.float32),
            "pmax": np.tile(s.maxmask[c * s.GPC : (c + 1) * s.GPC], (128, 1)).astype(np.float32),
        }
        maps.append(m)
    return maps


# ---------------------------------------------------------------- bass build
class _Bacc(bacc.Bacc):
    """Bacc whose act-table pass loads ONE set covering every activation
    used (Ln/Exp/Relu all live in natural_log_exp_and_others), instead of
    the per-instruction greedy choice that thrashes 84 table reloads when
    Ln and Exp interleave per block group."""

    def insert_act_table_loads(self):
        from concourse.hw_specs import get_activation_tables

        used = {
            i.func
            for b in self.main_func.blocks
            for i in b.instructions
            if isinstance(i, mybir.InstActivation)
        }
        if not used:
            return
        tables = list(get_activation_tables(self.m.arch).items())
        for idx, (name, fs) in enumerate(tables):
            if used <= fs:
                ld = mybir.InstLoadActFuncSet(
                    name=self.get_next_instruction_name(),
                    engine=mybir.EngineType.Activation,
                    act_func_set_id=idx,
                    ins=[],
                    outs=[],
                )
                self.register_instruction(ld)
                first_act_block = None
                for blk in self.main_func.blocks:
                    if any(
                        isinstance(i, mybir.InstActivation) for i in blk.instructions
                    ):
                        first_act_block = blk
                        break
                assert first_act_block is not None
                first_act_block.instructions.insert(0, ld)
                return
        super().insert_act_table_loads()


def build_nc(s, f):
    NB, NSLOT, NBLK, SG, GPC = s.NB, s.NSLOT, s.NBLK, s.SG, s.GPC
    NSC = NB // CHUNK_BINS
    NT_CH_A = CHUNK_BINS * TA            # tiles per A-chunk (48)
    NT_CH_B = CHUNK_BINS * TB
    NIDX_A = NT_CH_A * 128
    NIDX_B = NT_CH_B * 128
    NTA, NTB = NB * TA, NB * TB

    nc = _Bacc(
        get_trn_type() or "TRN2",
        num_devices=NCORES,
        num_swdge_queues=NQUEUES,
        dynamic_dma_scratch_size=DMA_SCRATCH,
    )

    # ---- I/O ----
    xT_d = nc.dram_tensor("xT", [F_IN, NSLOT], BF16, kind="ExternalInput")
    idxA_d = nc.dram_tensor("idxA", [128, NTA * 8], I16, kind="ExternalInput")
    idxB_d = nc.dram_tensor("idxB", [128, NTB * 8], I16, kind="ExternalInput")
    dstA16_d = nc.dram_tensor("dstA16", [128, NTA], BF16, kind="ExternalInput")
    dstB16_d = nc.dram_tensor("dstB16", [128, NTB], BF16, kind="ExternalInput")
    dinv_d = nc.dram_tensor("dinv", [128, NBLK], F32, kind="ExternalInput")
    mask_d = nc.dram_tensor("mask", [128, NBLK], F32, kind="ExternalInput")
    gidxm_d = nc.dram_tensor("gidxm", [128, GPC * SG // 16], I16, kind="ExternalInput")
    gidxx_d = nc.dram_tensor("gidxx", [128, GPC * SG // 16], I16, kind="ExternalInput")
    pminv_d = nc.dram_tensor("pminv", [128, GPC], F32, kind="ExternalInput")
    pmax_d = nc.dram_tensor("pmax", [128, GPC], F32, kind="ExternalInput")
    out_d = nc.dram_tensor("out", [s.G, 1], F32, kind="ExternalOutput")

    # ---- shared consts ----
    it = nc.inline_tensor
    Wc_d = it(f["Wc"].astype(NPBF), "Wc")                        # [5,64]
    btotb_d = it(np.tile(f["btot_conv"], (128, 1)), "btotb")     # [128,64]
    W1_d = [it(f["W1"][i], f"W1_{i}") for i in range(L)]         # [64,128]
    W2_d = [it(f["W2"][i], f"W2_{i}") for i in range(L)]         # [128,64]
    b1_d = [it(f["b1tot"][i][:, None], f"b1_{i}") for i in range(L)]   # [128,1]
    b2b_d = [it(np.tile(f["b2"][i], (128, 1)), f"b2b_{i}") for i in range(L)]
    gbb_d = [it(np.tile(f["ln_g"][i], (128, 1)), f"gbb_{i}") for i in range(L)]
    bbb_d = [it(np.tile(f["ln_b"][i], (128, 1)), f"bbb_{i}") for i in range(L)]
    abb_d = [it(np.tile(f["prelu_a"][i], (128, 1)), f"abb_{i}") for i in range(L)]
    l1W_d = [it(np.ascontiguousarray(f["lin1_W"][k * 128 : (k + 1) * 128]).astype(NPBF), f"l1W_{k}") for k in range(4)]
    l1b_d = it(f["lin1_b"][:, None], "l1b")                      # [128,1]
    l2W_d = it(f["lin2_W"].astype(NPBF), "l2W")                  # [128,64]
    l2b_d = it(f["lin2_b"][:, None], "l2b")                      # [64,1]
    oW_d = it(f["out_W"].astype(NPBF), "oW")                     # [64,1]
    iotab_d = it(np.tile(np.arange(32, dtype=np.float32), (128, 1)).astype(NPBF), "iotab")
    ident_d = it(np.eye(128, dtype=np.float32), "ident")

    # ---- internal DRAM ----
    ag_in = nc.dram_tensor("ag_in", [NSLOT, 2 * H], BF16)
    ag_out = nc.dram_tensor("ag_out", [NCORES * NSLOT, 2 * H], BF16, addr_space="Shared")
    pool_in = nc.dram_tensor("pool_in", [4, 128, GPC], BF16)
    pool_out = nc.dram_tensor("pool_out", [NCORES, 4, 128, GPC], BF16, addr_space="Shared")

    RG = [list(range(NCORES))]

    def allgather(cin, cout):
        if MOCK_COLLECTIVES:
            nc.sync.dma_start(out=cout[0 : cin.shape[0]], in_=cin[:])
        else:
            nc.gpsimd.collective_compute(
                "AllGather", ALU.bypass, replica_groups=RG,
                ins=[cin[:]], outs=[cout[:]],
            )

    with tile.TileContext(nc) as tc:
        with tc.tile_pool(name="persist", bufs=1) as pp:
            # conv-phase inputs load FIRST: the conv compute (which gates the
            # first AllGather and thus the whole pipeline) must not queue
            # behind the 15us of idx-array loads on the in-order SP queue
            cvs_ctx = tc.tile_pool(name="cvs", bufs=1)
            cvs = cvs_ctx.__enter__()
            xt_all = cvs.tile([F_IN, NSLOT], BF16, tag="xt_all")
            nc.sync.dma_start(out=xt_all[:], in_=xT_d[:, :])
            Wc = pp.tile([F_IN, H], BF16)
            nc.sync.dma_start(out=Wc[:], in_=Wc_d[:, :])
            dinv = pp.tile([128, NBLK], F32)
            nc.sync.dma_start(out=dinv[:], in_=dinv_d[:, :])

            # resident per-core data
            idxA_sb = pp.tile([128, NTA * 8], I16)
            nc.sync.dma_start(out=idxA_sb[:], in_=idxA_d[:, :])
            idxB_sb = pp.tile([128, NTB * 8], I16)
            nc.sync.dma_start(out=idxB_sb[:], in_=idxB_d[:, :])
            dstA16 = pp.tile([128, NTA], BF16)
            nc.sync.dma_start(out=dstA16[:], in_=dstA16_d[:, :])
            dstB16 = pp.tile([128, NTB], BF16)
            nc.sync.dma_start(out=dstB16[:], in_=dstB16_d[:, :])
            mask = pp.tile([128, NBLK], F32)
            nc.sync.dma_start(out=mask[:], in_=mask_d[:, :])

            # consts
            _ldn = [0]

            def ld(dram, shape, dtype=F32):
                _ldn[0] += 1
                nm = f"c{_ldn[0]}_{dram.name}"
                t = pp.tile(shape, dtype, name=nm, tag=nm)
                nc.sync.dma_start(out=t[:], in_=dram[tuple(slice(None) for _ in shape)])
                return t

            btotb = ld(btotb_d, [128, H])
            W1 = [ld(W1_d[i], [H, 2 * H]) for i in range(L)]
            W2 = [ld(W2_d[i], [2 * H, H]) for i in range(L)]
            b1 = [ld(b1_d[i], [128, 1]) for i in range(L)]
            b2b = [ld(b2b_d[i], [128, H]) for i in range(L)]
            gbb = [ld(gbb_d[i], [128, H]) for i in range(L)]
            bbb = [ld(bbb_d[i], [128, H]) for i in range(L)]
            abb = [ld(abb_d[i], [128, H]) for i in range(L)]
            l1W = [ld(l1W_d[k], [128, 128], BF16) for k in range(4)]
            l1b = ld(l1b_d, [128, 1])
            l2W = ld(l2W_d, [128, H], BF16)
            l2b = ld(l2b_d, [H, 1])
            oW = ld(oW_d, [H, 1], BF16)
            iotab = ld(iotab_d, [128, 32], BF16)
            ident = ld(ident_d, [128, 128])
            epsb = pp.tile([128, 1], F32)
            nc.vector.memset(epsb[:], EPS_BN)
            zb = pp.tile([128, 1], F32)
            nc.vector.memset(zb[:], 0.0)
            # pooling-phase inputs, prefetched at startup so the pool phase
            # doesn't serialize on their loads
            gim = pp.tile([128, GPC * SG // 16], I16)
            nc.sync.dma_start(out=gim[:], in_=gidxm_d[:, :])
            gix = pp.tile([128, GPC * SG // 16], I16)
            nc.sync.dma_start(out=gix[:], in_=gidxx_d[:, :])
            pscm = pp.tile([128, GPC], F32)
            nc.sync.dma_start(out=pscm[:], in_=pminv_d[:, :])
            pscx = pp.tile([128, GPC], F32)
            nc.sync.dma_start(out=pscx[:], in_=pmax_d[:, :])

            # persistent state (ledger bf16: halves SBUF + is directly the
            # pooling gather source)
            ledger = pp.tile([128, NBLK, (L + 1) * H], BF16)
            usc = pp.tile([128, NBLK, H], F32)       # h0n during conv, u in GEN
            ab = pp.tile([128, NBLK, 2 * H], BF16)

            assert NIDX_A == NIDX_B
            nidx_subreg = nc.gpsimd.to_reg(NIDX_A // GATHER_SPLIT)

            NGRP = -(-NBLK // 4)

            def edge_phase(tag, table_dram, drain_fn, group_fn=None):
                """Shared edge machinery (bf16 table, 2H channels).
                drain_fn(blk, psum_tile, ep, mpp); group_fn(g, ep) runs after
                blocks [4g, 4g+4) have drained (used to interleave the next
                layer's node phase into this phase's pipeline)."""
                nch = 2 * H
                BPS = CHUNK_BINS * BINCAP // 128   # blocks per superchunk
                with (
                    tc.tile_pool(name=f"ep_{tag}", bufs=1) as ep,
                    tc.tile_pool(name=f"epp_{tag}", bufs=3, space="PSUM") as epp,
                    tc.tile_pool(name=f"mpp_{tag}", bufs=1, space="PSUM") as mpp,
                ):
                    def build_sel(sc):
                        sa = ep.tile([128, NT_CH_A, 32], BF16, tag="sa", bufs=2)
                        nc.vector.tensor_tensor(
                            out=sa[:],
                            in0=dstA16[:, sc * NT_CH_A : (sc + 1) * NT_CH_A]
                            .unsqueeze(2).broadcast_to([128, NT_CH_A, 32]),
                            in1=iotab[:].unsqueeze(1)
                            .broadcast_to([128, NT_CH_A, 32]),
                            op=ALU.is_equal,
                        )
                        sb = ep.tile([128, NT_CH_B, 32], BF16, tag="sb", bufs=2)
                        nc.vector.tensor_tensor(
                            out=sb[:],
                            in0=dstB16[:, sc * NT_CH_B : (sc + 1) * NT_CH_B]
                            .unsqueeze(2).broadcast_to([128, NT_CH_B, 32]),
                            in1=iotab[:].unsqueeze(1)
                            .broadcast_to([128, NT_CH_B, 32]),
                            op=ALU.is_equal,
                        )
                        return sa, sb

                    sel = build_sel(0)
                    for sc in range(NSC):
                        ia = idxA_sb[:, sc * (NIDX_A // 16) : (sc + 1) * (NIDX_A // 16)]
                        ib = idxB_sb[:, sc * (NIDX_B // 16) : (sc + 1) * (NIDX_B // 16)]
                        ga = ep.tile([128, NT_CH_A, nch], BF16, tag="ga", bufs=2)
                        gb = ep.tile([128, NT_CH_B, nch], BF16, tag="gb", bufs=2)
                        if EDGE_GATHER:
                            GS = GATHER_SPLIT
                            tpc = NT_CH_A // GS      # tiles per sub-call
                            nn = tpc * 128
                            for k in range(GS):
                                nc.gpsimd.dma_gather(
                                    ga[:, k * tpc : (k + 1) * tpc, :],
                                    table_dram[0 : s.SPLIT, :],
                                    ia[:, k * (nn // 16) : (k + 1) * (nn // 16)],
                                    nn, nidx_subreg, nch,
                                    queue_num=(2 * k) % NQUEUES,
                                )
                                nc.gpsimd.dma_gather(
                                    gb[:, k * tpc : (k + 1) * tpc, :],
                                    table_dram[s.SPLIT : 2 * s.SPLIT, :],
                                    ib[:, k * (nn // 16) : (k + 1) * (nn // 16)],
                                    nn, nidx_subreg, nch,
                                    queue_num=(2 * k + 1) % NQUEUES,
                                )
                        else:
                            nc.vector.memset(ga[:], 0.25)
                            nc.vector.memset(gb[:], 0.25)
                        # sel prefetch: this superchunk's sel was built an
                        # iteration ago; build the next one now so the DVE
                        # queue serves it before this superchunk's drain and
                        # node-phase burst (keeps PE fed -> ga frees on time)
                        sa, sb = sel
                        if sc + 1 < NSC:
                            sel = build_sel(sc + 1)
                        for bl in range(BPS):
                            blk = sc * BPS + bl
                            ps = epp.tile([128, nch], F32, tag="eps", space="PSUM")
                            if EDGE_MM:
                                for j in range(4):
                                    lbin = bl * 4 + j   # bin within superchunk
                                    for t in range(TA):
                                        nc.tensor.matmul(
                                            out=ps[32 * j : 32 * j + 32, :],
                                            lhsT=sa[:, lbin * TA + t, :],
                                            rhs=ga[:, lbin * TA + t, :],
                                            start=(t == 0),
                                            stop=False,
                                            tile_position=(0, 32 * j),
                                        )
                                    for t in range(TB):
                                        nc.tensor.matmul(
                                            out=ps[32 * j : 32 * j + 32, :],
                                            lhsT=sb[:, lbin * TB + t, :],
                                            rhs=gb[:, lbin * TB + t, :],
                                            start=False,
                                            stop=(t == TB - 1),
                                            tile_position=(0, 32 * j),
                                        )
                            else:
                                nc.vector.memset(ps[:], 0.0)
                            drain_fn(blk, ps, ep, mpp)
                            if group_fn is not None:
                                # per-4-block groups, except the last 4 blocks
                                # which go per-block so the phase tail only
                                # carries one block's node chain
                                if blk >= NBLK - 4:
                                    group_fn(blk, 1, ep)
                                elif blk % 4 == 3:
                                    group_fn(blk - 3, 4, ep)

            # ================= conv =================
            # conv table rides the bf16 2H machinery: ab[:, :, 0:H] = h0*dinv,
            # ab[:, :, H:2H] = zeros (one upfront memset; GEN layers overwrite
            # the whole buffer afterwards).
            nc.vector.memset(ab[:].rearrange("p b c -> p (b c)"), 0.0)
            ag_in_pbc = ag_in.ap().rearrange("(p b) c -> p b c", p=128)
            with tc.tile_pool(name="cvp", bufs=2, space="PSUM") as cvp:
                for g in range(NGRP):
                    b0 = 4 * g
                    gn = min(4, NBLK - b0)
                    h0ps = cvp.tile([128, 4, H], F32, space="PSUM")
                    for b in range(gn):
                        nc.tensor.matmul(
                            out=h0ps[:, b, :],
                            lhsT=xt_all[:, (b0 + b) * 128 : (b0 + b + 1) * 128],
                            rhs=Wc[:],
                            start=True, stop=True,
                        )
                    nc.vector.tensor_tensor(
                        out=usc[:, b0 : b0 + gn, :], in0=h0ps[:, 0:gn, :],
                        in1=dinv[:, b0 : b0 + gn].unsqueeze(2)
                        .broadcast_to([128, gn, H]),
                        op=ALU.mult,
                    )
                    nc.vector.tensor_copy(
                        out=ab[:, b0 : b0 + gn, 0:H], in_=usc[:, b0 : b0 + gn, :]
                    )
                    nc.sync.dma_start(
                        out=ag_in_pbc[:, b0 : b0 + gn, :],
                        in_=ab[:, b0 : b0 + gn, :],
                    )
            # close the x-input pool before the edge pools open so the
            # 14KB/partition tile doesn't stack with gather buffers
            cvs_ctx.__exit__(None, None, None)
            if CONV_AG:
                allgather(ag_in, ag_out)

            def make_node_group(i):
                """Group callback computing layer i's LN/PReLU/exp table for
                blocks [4g, 4g+4) from ledger[:, :, i*H:(i+1)*H], writing
                usc (u, the root-add operand) and ab/ag_in (the edge table)."""
                tg = float(f["gen_t"][i])

                def node_group(b0, gn, ep):
                    led = ledger[:, b0 : b0 + gn, i * H : (i + 1) * H]
                    st = ep.tile([128, 4, 6], F32, tag="nst", bufs=2)
                    mv = ep.tile([128, 4, 2], F32, tag="nmv", bufs=2)
                    for b in range(gn):
                        nc.vector.bn_stats(out=st[:, b, :], in_=led[:, b, :])
                        nc.vector.bn_aggr(out=mv[:, b, :], in_=st[:, b, :])
                    # rstd = exp(-0.5*ln(var+eps)): Ln/Exp/Relu share one ACT
                    # table set, so no LoadActFuncSet thrash in the pipeline
                    rstd = ep.tile([128, 4], F32, tag="nrstd", bufs=2)
                    nc.scalar.activation(
                        out=rstd[:, 0:gn], in_=mv[:, 0:gn, 1], func=AF.Ln,
                        bias=epsb[:], scale=1.0,
                    )
                    nc.scalar.activation(
                        out=rstd[:, 0:gn], in_=rstd[:, 0:gn], func=AF.Exp,
                        bias=zb[:], scale=-0.5,
                    )
                    nmr = ep.tile([128, 4], F32, tag="nnmr", bufs=2)
                    nc.vector.tensor_tensor(
                        out=nmr[:, 0:gn], in0=mv[:, 0:gn, 0], in1=rstd[:, 0:gn],
                        op=ALU.mult,
                    )
                    u2 = ep.tile([128, 4, H], F32, tag="nu2", bufs=2)
                    for b in range(gn):
                        nc.vector.tensor_scalar(
                            out=u2[:, b, :], in0=led[:, b, :],
                            scalar1=rstd[:, b : b + 1], scalar2=nmr[:, b : b + 1],
                            op0=ALU.mult, op1=ALU.subtract,
                        )
                    u2g = u2[:, 0:gn, :]
                    bc = [128, gn, H]
                    nc.vector.tensor_tensor(
                        out=u2g, in0=u2g,
                        in1=gbb[i][:].unsqueeze(1).broadcast_to(bc), op=ALU.mult,
                    )
                    nc.vector.tensor_tensor(
                        out=u2g, in0=u2g,
                        in1=bbb[i][:].unsqueeze(1).broadcast_to(bc), op=ALU.add,
                    )
                    r4 = ep.tile([128, 4, H], F32, tag="nr4", bufs=2)
                    nc.vector.tensor_scalar(
                        out=r4[:, 0:gn, :], in0=u2g, scalar1=0.0, scalar2=None,
                        op0=ALU.max,
                    )
                    m4 = ep.tile([128, 4, H], F32, tag="nm4", bufs=2)
                    nc.vector.tensor_scalar(
                        out=m4[:, 0:gn, :], in0=u2g, scalar1=0.0, scalar2=None,
                        op0=ALU.min,
                    )
                    nc.vector.tensor_tensor(
                        out=m4[:, 0:gn, :], in0=m4[:, 0:gn, :],
                        in1=abb[i][:].unsqueeze(1).broadcast_to(bc), op=ALU.mult,
                    )
                    nc.vector.tensor_tensor(
                        out=usc[:, b0 : b0 + gn, :], in0=r4[:, 0:gn, :],
                        in1=m4[:, 0:gn, :], op=ALU.add,
                    )
                    # A = exp(t*(r+eps)), B = v*A ~= r*A (eps negligible)
                    A4 = ep.tile([128, 4, H], F32, tag="nA4", bufs=2)
                    nc.scalar.activation(
                        out=A4[:, 0:gn, :], in_=r4[:, 0:gn, :], func=AF.Exp,
                        bias=zb[:], scale=tg,
                    )
                    nc.vector.tensor_copy(out=ab[:, b0 : b0 + gn, 0:H], in_=A4[:, 0:gn, :])
                    nc.vector.tensor_tensor(
                        out=ab[:, b0 : b0 + gn, H : 2 * H], in0=r4[:, 0:gn, :],
                        in1=A4[:, 0:gn, :], op=ALU.mult,
                    )
                    nc.sync.dma_start(
                        out=ag_in_pbc[:, b0 : b0 + gn, :], in_=ab[:, b0 : b0 + gn, :]
                    )

                return node_group

            def conv_drain(blk, ps, ep, mpp):
                t1 = ep.tile([128, H], F32, tag="cd", bufs=3)
                nc.vector.tensor_add(t1[:], ps[:, 0:H], usc[:, blk, :])
                nc.vector.tensor_scalar(
                    out=t1[:], in0=t1[:],
                    scalar1=dinv[:, blk : blk + 1], scalar2=None, op0=ALU.mult,
                )
                nc.vector.tensor_add(t1[:], t1[:], btotb[:])
                nc.vector.tensor_scalar(
                    out=ledger[:, blk, 0:H], in0=t1[:],
                    scalar1=0.0, scalar2=mask[:, blk : blk + 1],
                    op0=ALU.max, op1=ALU.mult,
                )

            NGEN = L if PHASES >= 2 else 0
            if CONV_EDGE:
                edge_phase(
                    "cv", ag_out, conv_drain,
                    group_fn=make_node_group(0) if NGEN else None,
                )
            else:
                nc.vector.tensor_copy(
                    out=ledger[:, :, 0:H], in_=usc[:],
                )

            # ================= GEN layers =================
            for i in range(NGEN):
                allgather(ag_in, ag_out)

                def gen_drain(blk, ps, ep, mpp, i=i):
                    sden = ep.tile([128, H], F32, tag="sden", bufs=3)
                    nc.vector.tensor_scalar(
                        out=sden[:], in0=ps[:, 0:H], scalar1=1e-30, scalar2=None,
                        op0=ALU.add,
                    )
                    nc.vector.reciprocal(out=sden[:], in_=sden[:])
                    agg = ep.tile([128, H], F32, tag="agg", bufs=3)
                    nc.vector.tensor_tensor(
                        out=agg[:], in0=ps[:, H : 2 * H], in1=sden[:], op=ALU.mult
                    )
                    nc.vector.tensor_add(agg[:], agg[:], usc[:, blk, :])
                    tps = mpp.tile([H, 128], F32, tag="tps", space="PSUM")
                    nc.tensor.transpose(out=tps[:], in_=agg[:], identity=ident[:])
                    aggT = ep.tile([H, 128], F32, tag="aggT", bufs=3)
                    nc.vector.tensor_copy(out=aggT[:], in_=tps[:])
                    z1ps = mpp.tile([128, 128], F32, tag="z1", space="PSUM")
                    nc.tensor.matmul(
                        out=z1ps[:], lhsT=W1[i][:], rhs=aggT[:], start=True, stop=True
                    )
                    z1r = ep.tile([128, 128], F32, tag="z1r", bufs=3)
                    nc.scalar.activation(
                        out=z1r[:], in_=z1ps[:], func=AF.Relu, bias=b1[i][:], scale=1.0
                    )
                    z2ps = mpp.tile([128, H], F32, tag="z2", space="PSUM")
                    nc.tensor.matmul(
                        out=z2ps[:], lhsT=z1r[:], rhs=W2[i][:], start=True, stop=True
                    )
                    t2 = ep.tile([128, H], F32, tag="t2", bufs=3)
                    nc.vector.tensor_add(t2[:], z2ps[:], b2b[i][:])
                    nc.vector.tensor_add(t2[:], t2[:], ledger[:, blk, i * H : (i + 1) * H])
                    nc.vector.tensor_scalar(
                        out=ledger[:, blk, (i + 1) * H : (i + 2) * H], in0=t2[:],
                        scalar1=mask[:, blk : blk + 1], scalar2=None, op0=ALU.mult,
                    )
                    if i == NGEN - 1 and blk == 0 and PHASES >= 3:
                        # slot 0 is the pooling max-pad token
                        nc.vector.memset(ledger[0:1, 0, 0 : (L + 1) * H], -3.0e38)

                edge_phase(
                    f"g{i}", ag_out, gen_drain,
                    group_fn=make_node_group(i + 1) if i < NGEN - 1 else None,
                )

            # ================= pooling + head =================
            if PHASES < 3:
                dbg = nc.dram_tensor("dbg", [128, NBLK, (L + 1) * H], F32,
                                     kind="ExternalOutput")
                nc.sync.dma_start(out=dbg[:, :, :], in_=ledger[:])
            CH = (L + 1) * H
            from contextlib import ExitStack as _ES
            with _ES() as _pool_ctx:
              if PHASES >= 3:
                qp = _pool_ctx.enter_context(tc.tile_pool(name="pool", bufs=1))
                qpp = _pool_ctx.enter_context(
                    tc.tile_pool(name="poolps", bufs=2, space="PSUM")
                )
                gnidx_reg = nc.gpsimd.to_reg(2 * SG)
                ledflat = ledger[:].rearrange("p b c -> p (b c)")
                pooled = qp.tile([128, 4, GPC], BF16)
                PGS = 2 * SG                      # idxs per sub-call (2 graphs)
                # interleave mean/max chunks: gathers pipeline on both queues,
                # sum-reduces run on DVE, max-reduces on gpsimd in parallel
                for k in range(GPC // 2):
                    for which, gi, psc in ((0, gim, pscm), (1, gix, pscx)):
                        grid = qp.tile(
                            [128, 2, PGS], BF16, tag=f"grid{which}", bufs=3
                        )
                        nc.gpsimd.dma_gather(
                            grid[:, :, :],
                            ledflat,
                            gi[:, k * (PGS // 16) : (k + 1) * (PGS // 16)],
                            PGS, gnidx_reg, CH,
                            transpose=True,
                            sbuf_tokens_per_rank=128,
                            sbuf_free_dim_per_rank=CH * 2,
                            queue_num=(2 * k + which) % NQUEUES,
                        )
                        for half in range(2):
                            red = qp.tile([128, 2], F32, tag=f"red{which}", bufs=3)
                            red_op = (
                                nc.vector.reduce_sum
                                if which == 0
                                else nc.vector.reduce_max
                            )
                            red_op(
                                out=red[:],
                                in_=grid[:, half, :].rearrange(
                                    "p (m t) -> p m t", t=SG
                                ),
                                axis=mybir.AxisListType.X,
                            )
                            nc.vector.tensor_tensor(
                                out=pooled[:, which * 2 + half, 2 * k : 2 * k + 2],
                                in0=red[:],
                                in1=psc[:, 2 * k : 2 * k + 2], op=ALU.mult,
                            )
                nc.sync.dma_start(
                    out=pool_in.ap().rearrange("k p g -> p k g"), in_=pooled[:]
                )
                if MOCK_COLLECTIVES:
                    nc.sync.dma_start(
                        out=pool_out[0, :, :, :], in_=pool_in[:, :, :]
                    )
                else:
                    nc.gpsimd.collective_compute(
                        "AllGather", ALU.bypass, replica_groups=RG,
                        ins=[pool_in[:, :, :]], outs=[pool_out[:, :, :, :]],
                    )
                # head
                hps = qpp.tile([128, s.G], F32, tag="hps", space="PSUM")
                pk = []
                for k in range(4):
                    t = qp.tile([128, NCORES, GPC], BF16, tag=f"pk{k}")
                    nc.sync.dma_start(
                        out=t[:], in_=pool_out[:, k, :, :].rearrange("r p g -> p r g")
                    )
                    pk.append(t)
                for k in range(4):
                    nc.tensor.matmul(
                        out=hps[:], lhsT=l1W[k][:],
                        rhs=pk[k][:].rearrange("p r g -> p (r g)"),
                        start=(k == 0), stop=(k == 3),
                    )
                hz1 = qp.tile([128, s.G], BF16)
                nc.scalar.activation(
                    out=hz1[:], in_=hps[:], func=AF.Relu, bias=l1b[:], scale=1.0
                )
                h2ps = qpp.tile([H, s.G], F32, tag="h2ps", space="PSUM")
                nc.tensor.matmul(out=h2ps[:], lhsT=l2W[:], rhs=hz1[:], start=True, stop=True)
                hz2 = qp.tile([H, s.G], BF16)
                nc.scalar.activation(
                    out=hz2[:], in_=h2ps[:], func=AF.Relu, bias=l2b[:], scale=1.0
                )
                ops = qpp.tile([1, s.G], F32, tag="ops", space="PSUM")
                nc.tensor.matmul(out=ops[:], lhsT=oW[:], rhs=hz2[:], start=True, stop=True)
                osb = qp.tile([1, s.G], F32)
                nc.vector.tensor_scalar(
                    out=osb[:], in0=ops[:], scalar1=float(f["out_b"][0]),
                    scalar2=None, op0=ALU.add,
                )
                nc.sync.dma_start(out=out_d.ap().rearrange("g one -> one g"), in_=osb[:])

    nc.compile()
    return nc


def _insert_library_loads(nc):
    import bass_rust as _bass_rust
    from concourse.library_config import all_libraries, standard

    inst_type_to_lib_mask = {}
    for lib in all_libraries:
        for inst_type in lib.instructions:
            inst_type_to_lib_mask[inst_type] = inst_type_to_lib_mask.get(
                inst_type, 0
            ) | (1 << lib.index)
    _bass_rust.insert_library_loads(
        nc, inst_type_to_lib_mask, len(all_libraries), standard.index
    )


# ---------------------------------------------------------------- wait split
def split_waits(nc, max_waits: int = 1) -> int:
    nsplit = 0
    for fn in nc.m.functions:
        for bb in fn.blocks:
            new_insts = []
            for ins in bb.instructions:
                si = ins.sync_info
                if si is not None and si.on_wait and len(si.on_wait) > max_waits:
                    waits = list(si.on_wait)
                    spill, keep = waits[:-max_waits], waits[-max_waits:]
                    for k, w in enumerate(spill):
                        nop = mybir.InstNoOp(
                            name=f"{ins.name}-wsplit{k}",
                            engine=ins.engine,
                            bass_nofuse=True,
                            sync_info=mybir.SyncInfo(on_wait=[w], on_update=[]),
                        )
                        new_insts.append(nop)
                        nc.register_instruction(nop, overwrite=True)
                        nsplit += 1
                    si.on_wait = keep
                new_insts.append(ins)
            if len(new_insts) != len(bb.instructions):
                bb.instructions[:] = new_insts
    return nsplit


# ---------------------------------------------------------------- entry
def kernel(**inputs) -> np.ndarray:
    x = np.asarray(inputs["x"], np.float32)
    ei = np.asarray(inputs["edge_index"], np.int64)
    bi = np.asarray(inputs["batch_idx"], np.int64)
    G = 256
    s = build_schedule(ei, bi, G)
    f = fold_weights(inputs)
    maps = build_inmaps(s, x)
    nc = build_nc(s, f)
    res = run_bass_kernel_spmd(nc, maps, core_ids=list(range(NCORES)))
    return np.asarray(res.results[0]["out"], np.float32)



# revision 3
# speedup vs baseline: 1.0105x; 1.0105x over previous
"""Trainium2 Bass kernel for nn_GCN_5403068858882 (GCN + 3x GENConv + pool head).

v2 rewrite of the staged baseline. Key changes vs baseline (1.156 ms):
- Conv rides x-space: the conv table is HOST-STAGED (x*dinv padded into 256B
  rows, global over all cores) so conv needs no device table write and no
  AllGather; gathers read only 64B/edge (descriptor floor) and the 5->64
  W-matmul runs after aggregation (per-block, not per-edge-channel).
- GEN tables store [A-1 | B/8] (A=exp(t*relu(u)), B=relu(u)*A): the drain adds
  the known in-degree back (SA = indeg + sum(A-1)) which keeps fp8 precision
  for layers 1-2 (128-byte rows via elem_size<stride raw gathers = half the
  per-descriptor DMA cost). Layer 0 stays bf16 (fp8 there fails the rel-err
  gate); recip is exp(-ln(x)) on the otherwise-idle ACT engine.
- 3072-idx gather calls (SWDGE ring = 64KB scratch) cut Pool-engine prep from
  209us to ~106us/phase; per-call uniform trailing pad indices are -1
  (descriptor-skipped), with the valid count maxed across cores.
- Drains/node phases lean on ACT (Ln/Exp/Relu/Copy with per-partition
  scale/bias APs) and bf16 DVE ops; ledger mask is gone (invalid slots stay
  exactly 0 by induction).
- Mean-pool is a gsel matmul over ledger blocks (PE) instead of SBUF gathers;
  only max-pool still gathers. Head is f32 (precision headroom for fp8).
"""

import numpy as np
import ml_dtypes

import concourse.bass as bass
import concourse.bacc as bacc
import concourse.mybir as mybir
import concourse.tile as tile
import concourse.ap_utils as ap_utils
from concourse.bass_utils import run_bass_kernel_spmd
from concourse.bass import round_up_to_multiple, exact_div
from concourse._compat import get_trn_type

F32 = mybir.dt.float32
BF16 = mybir.dt.bfloat16
FP8 = mybir.dt.float8e4
I16 = mybir.dt.int16
AF = mybir.ActivationFunctionType
ALU = mybir.AluOpType
NPBF = ml_dtypes.bfloat16
NPF8 = ml_dtypes.float8_e4m3fn

H = 64
F_IN = 5
L = 3
EPS_BN = 1e-5
NCORES = 8
BINCAP = 32
TPB = 3                   # 128-edge tiles per bin per side (cap = 384)
CAP = TPB * 128
SC_BINS = 8               # bins per superchunk -> 3072-idx calls
DMA_SCRATCH = 65536       # SWDGE ring bytes/partition (ring = /16 descriptors)
NQUEUES = 2
MOCK_COLLECTIVES = False
L12_FP8 = True            # layers 1-2 table dtype fp8 (128B rows)


# ---------------------------------------------------------------- raw gather
def raw_dma_gather(gp, out_ap, in_ap, idxs_ap, num_idxs, num_idxs_reg,
                   elem_size, elem_step, queue_num=0):
    """dma_gather without the elem_size_bytes%256 assert (row STRIDE must
    still be a 256B multiple - that is the descriptor format's constraint)."""
    gp._assert_queue_num(queue_num)
    assert idxs_ap.dtype == mybir.dt.int16
    assert in_ap.dtype == out_ap.dtype
    assert in_ap.space == bass.MemorySpace.DRAM
    assert ap_utils.ap_is_contiguous(in_ap.ap[1:])
    assert ap_utils.ap_is_contiguous(out_ap.ap[1:])
    assert ap_utils.ap_is_contiguous(idxs_ap.ap[1:])
    assert in_ap.ap[-1][1] == out_ap.ap[-1][1] == elem_size
    assert out_ap.ap[0][1] * out_ap.ap[1][1] == round_up_to_multiple(num_idxs, 128)
    assert in_ap.ap[0][0] == elem_step
    stride_bytes = elem_step * mybir.dt.size(in_ap.dtype)
    stride_bytes_256 = exact_div(stride_bytes, 256)
    assert stride_bytes_256 < 256
    _in_ap = gp.lower_ap_dma(in_ap, for_custom_bir_dma=True)
    return gp.add_instruction(
        mybir.InstDMAGatherAnt(
            name=gp.bass.get_next_instruction_name(),
            ins=[*_in_ap, gp.lower_ap(idxs_ap),
                 gp.lower_val_access(gp.to_reg(num_idxs_reg))],
            outs=[gp.lower_ap(out_ap)],
            transpose=False,
            num_idxs=num_idxs,
            elem_size=elem_size,
            stride_bytes_256=stride_bytes_256,
            gen_mode=0,
            single_packet=False,   # multi-packet: >1024-idx calls work on hw
            queue_num=queue_num,
            sbuf_tokens_per_rank=0,
            sbuf_free_dim_per_rank=0,
            sbuf_free_dim_pad_per_rank=0,
            sbuf_byte_offset=0,
        )
    )


# ---------------------------------------------------------------- schedule
class Sched:
    pass


def build_schedule(edge_index, batch_idx, G):
    s = Sched()
    src = np.asarray(edge_index[0], np.int64)
    dst = np.asarray(edge_index[1], np.int64)
    batch = np.asarray(batch_idx, np.int64)
    n = batch.shape[0]
    s.G = G
    s.GPC = GPC = G // NCORES

    indeg = np.bincount(dst, minlength=n)
    deg = indeg.astype(np.float64) + 1.0
    s.dinv_node = (deg ** -0.5).astype(np.float32)
    s.indeg_node = indeg

    a_edge = batch[src] < (G // 2)
    acnt = np.bincount(dst[a_edge], minlength=n)
    bcnt = np.bincount(dst[~a_edge], minlength=n)

    gstart = np.searchsorted(batch, np.arange(G))
    gend = np.searchsorted(batch, np.arange(G), side="right")
    s.cnt = cnt = gend - gstart

    core_bins = []
    for c in range(NCORES):
        lo, hi = gstart[c * GPC], gend[(c + 1) * GPC - 1]
        nodes = np.arange(lo, hi)
        order = nodes[np.argsort(-(acnt[nodes] + bcnt[nodes]), kind="stable")]
        # slots 0/1 of bin 0 reserved: pooling max/mean pad tokens
        bins_n, bins_a, bins_b = [[-1, -1]], [0], [0]
        for nd in order:
            a, b = int(acnt[nd]), int(bcnt[nd])
            placed = False
            for i in range(len(bins_n)):
                if (len(bins_n[i]) < BINCAP and bins_a[i] + a <= CAP
                        and bins_b[i] + b <= CAP):
                    bins_n[i].append(nd)
                    bins_a[i] += a
                    bins_b[i] += b
                    placed = True
                    break
            if not placed:
                bins_n.append([nd])
                bins_a.append(a)
                bins_b.append(b)
        core_bins.append(bins_n)

    NB = max(len(b) for b in core_bins)
    NB = -(-NB // (2 * SC_BINS)) * (2 * SC_BINS)
    s.NB = NB
    s.NSLOT = NSLOT = NB * BINCAP
    s.NBLK = NSLOT // 128
    s.NSC = NB // SC_BINS
    s.SPLIT = 4 * NSLOT
    assert s.SPLIT <= 32768, s.SPLIT

    slot2node = np.full((NCORES, NSLOT), -1, np.int64)
    pos_of_node = np.full(n, -1, np.int64)
    for c in range(NCORES):
        for bi, bn in enumerate(core_bins[c]):
            for j, nd in enumerate(bn):
                if nd >= 0:
                    slot2node[c, bi * BINCAP + j] = nd
                    pos_of_node[nd] = c * NSLOT + bi * BINCAP + j
    assert (pos_of_node >= 0).all()
    s.slot2node, s.pos_of_node = slot2node, pos_of_node

    # table rows are partition-major: local slot sl lives in DRAM row
    # core*NSLOT + (sl%128)*NBLK + sl//128
    sl = pos_of_node[src] % NSLOT
    src_pos = (pos_of_node[src] // NSLOT) * NSLOT + (sl % 128) * s.NBLK + sl // 128

    dst_pos = pos_of_node[dst]
    dst_core = dst_pos // NSLOT
    dst_bin = (dst_pos % NSLOT) // BINCAP
    dst_slot = (dst_pos % NSLOT) % BINCAP

    NPOS = NB * CAP
    idxA = np.zeros((NCORES, NPOS), np.int16)
    idxB = np.zeros((NCORES, NPOS), np.int16)
    dstA = np.full((NCORES, NPOS), -1.0, np.float32)
    dstB = np.full((NCORES, NPOS), -1.0, np.float32)
    lastA = np.zeros((NCORES, NB), np.int64)   # valid count per bin
    lastB = np.zeros((NCORES, NB), np.int64)

    order = np.lexsort((src_pos, dst_bin, dst_core))
    eo_src, eo_core = src_pos[order], dst_core[order]
    eo_bin, eo_slot, eo_a = dst_bin[order], dst_slot[order], a_edge[order]

    for c in range(NCORES):
        msk_c = eo_core == c
        for idxarr, dstarr, lastarr, off, grp in (
            (idxA, dstA, lastA, 0, True),
            (idxB, dstB, lastB, s.SPLIT, False),
        ):
            msk = msk_c & (eo_a == grp)
            bins_e, srcs, slots = eo_bin[msk], eo_src[msk] - off, eo_slot[msk]
            bs = np.searchsorted(bins_e, np.arange(NB))
            be = np.searchsorted(bins_e, np.arange(NB), side="right")
            for bi in range(NB):
                k = be[bi] - bs[bi]
                assert k <= CAP
                base = bi * CAP
                idxarr[c, base:base + k] = srcs[bs[bi]:be[bi]].astype(np.int16)
                dstarr[c, base:base + k] = slots[bs[bi]:be[bi]].astype(np.float32)
                lastarr[c, bi] = k

    # per-superchunk uniform valid counts: positions >= valid become -1
    # (descriptor-skipped); valid = max over cores so the reg is uniform.
    # Two window sizes: 8-bin (L0) and 16-bin (conv/L1/L2) superchunks.
    def valid_counts(scb):
        win = scb * CAP
        nsc = NB // scb
        vA = np.zeros(nsc, np.int64)
        vB = np.zeros(nsc, np.int64)
        for sc in range(nsc):
            b0 = sc * scb
            for lastarr, validarr in ((lastA, vA), (lastB, vB)):
                v = 0
                for c in range(NCORES):
                    for bi in range(b0 + scb - 1, b0 - 1, -1):
                        if lastarr[c, bi] > 0:
                            v = max(v, (bi - b0) * CAP + lastarr[c, bi])
                            break
                validarr[sc] = v
        return vA, vB

    s.validA8, s.validB8 = valid_counts(SC_BINS)
    s.validA16, s.validB16 = valid_counts(2 * SC_BINS)
    # -1 tails must respect the SMALLER window (8-bin): a 16-bin call sees
    # interior -1 runs... so instead tails are cut only at 8-bin windows and
    # the 16-bin call valid count is derived from them: a 16-bin window's
    # trailing -1 run is the second 8-bin window's tail (interior -1s from
    # the first 8-bin window would violate the trailing-only rule). Guard:
    # only apply the first window's tail cut when the second window is
    # entirely empty.
    WIN = SC_BINS * CAP
    for c in range(NCORES):
        for sc in range(s.NSC):
            a0 = sc * WIN
            vA = s.validA8[sc]
            vB = s.validB8[sc]
            if sc % 2 == 0 and s.validA8[sc + 1] > 0:
                vA = WIN
            if sc % 2 == 0 and s.validB8[sc + 1] > 0:
                vB = WIN
            idxA[c, a0 + vA:a0 + WIN] = -1
            idxB[c, a0 + vB:a0 + WIN] = -1
    # recompute the 16-bin valid counts to match the actual -1 layout
    for sc2 in range(NB // (2 * SC_BINS)):
        if s.validA8[2 * sc2 + 1] > 0:
            s.validA16[sc2] = WIN + s.validA8[2 * sc2 + 1]
        else:
            s.validA16[sc2] = s.validA8[2 * sc2]
        if s.validB8[2 * sc2 + 1] > 0:
            s.validB16[sc2] = WIN + s.validB8[2 * sc2 + 1]
        else:
            s.validB16[sc2] = s.validB8[2 * sc2]
    # L0 (8-bin windows) regs must match the per-core -1 layout exactly:
    # valid count per 8-bin window == count of non-negative entries.
    for sc in range(s.NSC):
        if sc % 2 == 0 and s.validA8[sc + 1] > 0:
            s.validA8[sc] = WIN
        if sc % 2 == 0 and s.validB8[sc + 1] > 0:
            s.validB8[sc] = WIN

    s.idxA, s.idxB, s.dstA, s.dstB = idxA, idxB, dstA, dstB
    s.tmaxA = np.ceil(lastA.max(axis=0) / 128).astype(np.int64)
    s.tmaxB = np.ceil(lastB.max(axis=0) / 128).astype(np.int64)

    valid = slot2node >= 0
    s.valid = valid
    s.dinv_slot = np.where(
        valid, s.dinv_node[np.clip(slot2node, 0, None)], 0.0
    ).astype(np.float32)
    s.indeg8_slot = np.where(
        valid,
        np.maximum(s.indeg_node[np.clip(slot2node, 0, None)], 1) / 8.0,
        0.125,
    ).astype(np.float32)

    # pooling
    maxcnt = int(cnt.max())
    SG = max(64, -(-maxcnt // 64) * 64)
    s.SG = SG
    gidx_max = np.zeros((NCORES, GPC * SG), np.int16)
    gsel = np.zeros((NCORES, 128, s.NBLK, GPC), np.float32)
    for c in range(NCORES):
        for gl in range(GPC):
            g = c * GPC + gl
            slots = (pos_of_node[np.arange(gstart[g], gend[g])] % NSLOT).astype(
                np.int64)
            base = gl * SG
            gidx_max[c, base:base + len(slots)] = slots.astype(np.int16)
            # remaining entries point at slot 0 (the -inf pad token)
            gsel[c, slots % 128, slots // 128, gl] = 1.0
    s.gidx_max = gidx_max
    s.gsel = gsel
    s.inv_cnt = (1.0 / np.maximum(cnt, 1)).astype(np.float32)
    s.maxmask = (cnt > 0).astype(np.float32)
    return s


def fold_weights(w):
    f = {}
    w32 = {k: np.asarray(v, np.float32) if np.asarray(v).dtype != np.int64 else v
           for k, v in w.items()}
    sbn1 = w32["bn1_g"] / np.sqrt(1.0 + EPS_BN)
    f["Wc"] = (w32["conv1_W"] * sbn1[None, :]).astype(np.float32)
    f["btot_conv"] = (w32["conv1_b"] * sbn1 + w32["bn1_b"]).astype(np.float32)
    f["ln_g"], f["ln_b"] = w32["ln_g"], w32["ln_b"]
    f["prelu_a"], f["gen_t"] = w32["prelu_a"], w32["gen_t"]
    f["ln_unit"] = (np.allclose(f["ln_g"], 1.0) and np.allclose(f["ln_b"], 0.0))
    f["alpha_const"] = [
        float(f["prelu_a"][i][0]) if np.allclose(f["prelu_a"][i], f["prelu_a"][i][0])
        else None for i in range(L)
    ]
    f["W1"], f["b1tot"], f["W2"], f["b2"] = [], [], [], []
    for i in range(L):
        smlp = w32["mlp_bn_g"][i] / np.sqrt(1.0 + EPS_BN)
        f["W1"].append((w32["mlp_W1"][i] * smlp[None, :]).astype(np.float32))
        f["b1tot"].append(
            (w32["mlp_b1"][i] * smlp + w32["mlp_bn_b"][i]).astype(np.float32))
        f["W2"].append(w32["mlp_W2"][i])
        f["b2"].append(w32["mlp_b2"][i])
    f["b1_zero"] = [bool(np.all(b == 0.0)) for b in f["b1tot"]]
    f["b2_zero"] = [bool(np.all(b == 0.0)) for b in f["b2"]]
    f["btot_zero"] = bool(np.all(f["btot_conv"] == 0.0))
    for k in ("lin1_W", "lin1_b", "lin2_W", "lin2_b", "out_W", "out_b"):
        f[k] = w32[k]
    return f


def _wrap16(arr):
    """[K*16] -> [128, K] gather-idx layout (i at [i%16, i//16], tiled x8)."""
    a = np.asarray(arr, np.int16).reshape(-1, 16).T
    return np.tile(a, (8, 1)).copy()


def _tile_major(arr, ntiles):
    """[ntiles*128] -> [128, ntiles] (partition = slot within tile)."""
    return np.ascontiguousarray(np.asarray(arr).reshape(ntiles, 128).T)


def build_inmaps(s, x, f):
    n = x.shape[0]
    NSLOT, NBLK, NB = s.NSLOT, s.NBLK, s.NB
    xd = np.asarray(x, np.float32) * s.dinv_node[:, None]        # [n, 5]

    # global conv table: row core*NSLOT + (sl%128)*NBLK + sl//128 = xd | 0pad
    ctab = np.zeros((NCORES * NSLOT, 128), np.float32)
    for c in range(NCORES):
        v = s.valid[c]
        sl = np.arange(NSLOT)[v]
        rows = c * NSLOT + (sl % 128) * NBLK + sl // 128
        ctab[rows, 0:F_IN] = xd[s.slot2node[c][v]]
    ctab = ctab.astype(NPBF)

    NT = NB * TPB        # 128-edge tiles per side
    maps = []
    for c in range(NCORES):
        xdT = np.zeros((F_IN, NSLOT), np.float32)
        v = s.valid[c]
        xdT[:, v] = xd[s.slot2node[c][v]].T
        # merged dst array: per sc, 24 A-cols then 24 B-cols
        dA = _tile_major(s.dstA[c], NT)          # [128, NT]
        dB = _tile_major(s.dstB[c], NT)
        dall = np.zeros((128, NB * 6), np.float32)
        for sc in range(s.NSC):
            dall[:, sc * 48:sc * 48 + 24] = dA[:, sc * 24:(sc + 1) * 24]
            dall[:, sc * 48 + 24:sc * 48 + 48] = dB[:, sc * 24:(sc + 1) * 24]
        m = {
            "ctab": ctab,
            "xdT": np.ascontiguousarray(xdT).astype(NPBF),
            "idxA": _wrap16(s.idxA[c]),
            "idxB": _wrap16(s.idxB[c]),
            "dall": dall.astype(NPBF),
            "dinv": np.ascontiguousarray(s.dinv_slot[c].reshape(NBLK, 128).T),
            "indeg8": np.ascontiguousarray(
                s.indeg8_slot[c].reshape(NBLK, 128).T),
            "gsel": np.ascontiguousarray(
                s.gsel[c].reshape(128, NBLK * s.GPC)).astype(NPBF),
            "gidxx": _wrap16(s.gidx_max[c]),
            "pinv": s.inv_cnt[c * s.GPC:(c + 1) * s.GPC, None].copy(),
            "pmax": np.tile(s.maxmask[c * s.GPC:(c + 1) * s.GPC], (128, 1)),
        }
        maps.append(m)
    return maps


# ---------------------------------------------------------------- bass build
class _Bacc(bacc.Bacc):
    """Load ONE activation-table set covering every used function."""

    def insert_act_table_loads(self):
        from concourse.hw_specs import get_activation_tables

        used = {
            i.func
            for b in self.main_func.blocks
            for i in b.instructions
            if isinstance(i, mybir.InstActivation)
        }
        if not used:
            return
        tables = list(get_activation_tables(self.m.arch).items())
        for idx, (name, fs) in enumerate(tables):
            if used <= fs:
                ld = mybir.InstLoadActFuncSet(
                    name=self.get_next_instruction_name(),
                    engine=mybir.EngineType.Activation,
                    act_func_set_id=idx,
                    ins=[], outs=[],
                )
                self.register_instruction(ld)
                for blk in self.main_func.blocks:
                    if any(isinstance(i, mybir.InstActivation)
                           for i in blk.instructions):
                        blk.instructions.insert(0, ld)
                        return
                raise AssertionError
        super().insert_act_table_loads()


def build_nc(s, f):
    NB, NSLOT, NBLK, SG, GPC, NSC = s.NB, s.NSLOT, s.NBLK, s.SG, s.GPC, s.NSC
    NT = NB * TPB
    WIN = SC_BINS * CAP          # idx positions per superchunk per side
    CH = (L + 1) * H

    nc = _Bacc(
        get_trn_type() or "TRN2",
        num_devices=NCORES,
        num_swdge_queues=NQUEUES,
        dynamic_dma_scratch_size=DMA_SCRATCH,
    )

    # ---- I/O ----
    ctab_d = nc.dram_tensor("ctab", [NCORES * NSLOT, 128], BF16, kind="ExternalInput")
    xdT_d = nc.dram_tensor("xdT", [F_IN, NSLOT], BF16, kind="ExternalInput")
    idxA_d = nc.dram_tensor("idxA", [128, NB * CAP // 16], I16, kind="ExternalInput")
    idxB_d = nc.dram_tensor("idxB", [128, NB * CAP // 16], I16, kind="ExternalInput")
    dall_d = nc.dram_tensor("dall", [128, NB * 6], BF16, kind="ExternalInput")
    dinv_d = nc.dram_tensor("dinv", [128, NBLK], F32, kind="ExternalInput")
    indeg8_d = nc.dram_tensor("indeg8", [128, NBLK], F32, kind="ExternalInput")
    gsel_d = nc.dram_tensor("gsel", [128, NBLK * GPC], BF16, kind="ExternalInput")
    gidxx_d = nc.dram_tensor("gidxx", [128, GPC * SG // 16], I16, kind="ExternalInput")
    pinv_d = nc.dram_tensor("pinv", [GPC, 1], F32, kind="ExternalInput")
    pmax_d = nc.dram_tensor("pmax", [128, GPC], F32, kind="ExternalInput")
    out_d = nc.dram_tensor("out", [s.G, 1], F32, kind="ExternalOutput")

    # ---- consts ----
    it = nc.inline_tensor
    Wc5_d = it(f["Wc"].astype(NPBF), "Wc5")                      # [5, 64]
    W1_d = [it(f["W1"][i].astype(NPBF), f"W1_{i}") for i in range(L)]   # [64,128]
    W2_d = [it(f["W2"][i].astype(NPBF), f"W2_{i}") for i in range(L)]   # [128,64]
    b1_d = [None if f["b1_zero"][i] else it(f["b1tot"][i][:, None], f"b1_{i}")
            for i in range(L)]
    b2b_d = [None if f["b2_zero"][i] else
             it(np.tile(f["b2"][i], (128, 1)), f"b2b_{i}") for i in range(L)]
    btotb_d = None if f["btot_zero"] else it(np.tile(f["btot_conv"], (128, 1)), "btotb")
    gbb_d = [it(np.tile(f["ln_g"][i], (128, 1)), f"gbb_{i}") for i in range(L)] \
        if not f["ln_unit"] else [None] * L
    bbb_d = [it(np.tile(f["ln_b"][i], (128, 1)), f"bbb_{i}") for i in range(L)] \
        if not f["ln_unit"] else [None] * L
    abb_d = [None if f["alpha_const"][i] is not None else
             it(np.tile(f["prelu_a"][i], (128, 1)), f"abb_{i}") for i in range(L)]
    l1W_d = [it(np.ascontiguousarray(f["lin1_W"][k * 128:(k + 1) * 128]), f"l1W_{k}")
             for k in range(4)]                                   # [128,128] f32
    l1b_d = it(f["lin1_b"][:, None], "l1b")
    l2W_d = it(f["lin2_W"], "l2W")                                # [128,64] f32
    l2b_d = it(f["lin2_b"][:, None], "l2b")
    oW_d = it(f["out_W"], "oW")                                   # [64,1] f32
    iota_d = it(np.tile(np.arange(32, dtype=np.float32), (128, 1)).astype(NPBF), "iota")
    ident_d = it(np.eye(128, dtype=np.float32), "ident")

    # ---- internal DRAM ----
    tab0 = nc.dram_tensor("tab0", [NSLOT, 128], BF16)
    agg0 = nc.dram_tensor("agg0", [NCORES * NSLOT, 128], BF16, addr_space="Shared")
    T8DT = FP8 if L12_FP8 else BF16
    T8W = 256 if L12_FP8 else 128
    tab8 = nc.dram_tensor("tab8", [NSLOT, T8W], T8DT)
    agg8 = nc.dram_tensor("agg8", [NCORES * NSLOT, T8W], T8DT, addr_space="Shared")
    pool_in = nc.dram_tensor("pool_in", [4, 128, GPC], F32)
    pool_out = nc.dram_tensor("pool_out", [NCORES, 4, 128, GPC], F32,
                              addr_space="Shared")

    RG = [list(range(NCORES))]

    def allgather(cin, cout):
        if MOCK_COLLECTIVES:
            nc.sync.dma_start(out=cout[0:cin.shape[0]], in_=cin[:])
        else:
            nc.gpsimd.collective_compute(
                "AllGather", ALU.bypass, replica_groups=RG,
                ins=[cin[:]], outs=[cout[:]],
            )

    regs = {}

    def reg_of(v):
        if v not in regs:
            regs[v] = nc.gpsimd.to_reg(int(v))
        return regs[v]

    with tile.TileContext(nc) as tc:
        with tc.tile_pool(name="persist", bufs=1) as pp:
            # gather indexes first: conv gathers gate the pipeline
            idxA_sb = pp.tile([128, NB * CAP // 16], I16)
            nc.sync.dma_start(out=idxA_sb[:], in_=idxA_d[:, :])
            idxB_sb = pp.tile([128, NB * CAP // 16], I16)
            nc.sync.dma_start(out=idxB_sb[:], in_=idxB_d[:, :])
            dall = pp.tile([128, NB * 6], BF16)
            nc.sync.dma_start(out=dall[:], in_=dall_d[:, :])
            iota = pp.tile([128, 32], BF16)
            nc.sync.dma_start(out=iota[:], in_=iota_d[:, :])
            xdT = pp.tile([F_IN, NSLOT], BF16)
            nc.sync.dma_start(out=xdT[:], in_=xdT_d[:, :])
            Wc5 = pp.tile([F_IN, H], BF16)
            nc.sync.dma_start(out=Wc5[:], in_=Wc5_d[:, :])
            dinv = pp.tile([128, NBLK], F32)
            nc.sync.dma_start(out=dinv[:], in_=dinv_d[:, :])
            indeg8 = pp.tile([128, NBLK], F32)
            nc.sync.dma_start(out=indeg8[:], in_=indeg8_d[:, :])

            _ldn = [0]

            def ld(dram, shape, dtype=F32):
                _ldn[0] += 1
                nm = f"c{_ldn[0]}_{dram.name}"
                t = pp.tile(shape, dtype, name=nm, tag=nm)
                nc.sync.dma_start(out=t[:], in_=dram[tuple(slice(None) for _ in shape)])
                return t

            W1 = [ld(W1_d[i], [H, 128], BF16) for i in range(L)]
            W2 = [ld(W2_d[i], [128, H], BF16) for i in range(L)]
            b1 = [ld(b1_d[i], [128, 1]) if b1_d[i] is not None else None
                  for i in range(L)]
            b2b = [ld(b2b_d[i], [128, H]) if b2b_d[i] is not None else None
                   for i in range(L)]
            btotb = ld(btotb_d, [128, H]) if btotb_d is not None else None
            gbb = [ld(gbb_d[i], [128, H]) if gbb_d[i] is not None else None
                   for i in range(L)]
            bbb = [ld(bbb_d[i], [128, H]) if bbb_d[i] is not None else None
                   for i in range(L)]
            abb = [ld(abb_d[i], [128, H]) if abb_d[i] is not None else None
                   for i in range(L)]
            ident = ld(ident_d, [128, 128])
            l1W = [ld(l1W_d[k], [128, 128]) for k in range(4)]
            l1b = ld(l1b_d, [128, 1])
            l2W = ld(l2W_d, [128, H])
            l2b = ld(l2b_d, [H, 1])
            oW = ld(oW_d, [H, 1])
            gselt = pp.tile([128, NBLK * GPC], BF16)
            nc.sync.dma_start(out=gselt[:], in_=gsel_d[:, :])
            gix = pp.tile([128, GPC * SG // 16], I16)
            nc.sync.dma_start(out=gix[:], in_=gidxx_d[:, :])
            pinv = pp.tile([GPC, 1], F32)
            nc.sync.dma_start(out=pinv[:], in_=pinv_d[:, :])
            pmaxt = pp.tile([128, GPC], F32)
            nc.sync.dma_start(out=pmaxt[:], in_=pmax_d[:, :])
            epsb = pp.tile([128, 1], F32)
            nc.vector.memset(epsb[:], EPS_BN)

            # persistent state
            ledger = pp.tile([128, NBLK, CH], BF16)
            usc = pp.tile([128, NBLK, H], F32)

            tab0_pbc = tab0.ap().rearrange("(p b) c -> p b c", p=128)
            tab8_pbc = tab8.ap().rearrange("(p b) c -> p b c", p=128)

            # ---------------- node phase (builds layer-i edge table) -------
            def make_node_group(i):
                tg = float(f["gen_t"][i])
                tdt = BF16 if i == 0 else T8DT
                tpbc = tab0_pbc if i == 0 else tab8_pbc

                def node_group(b0, gn, ep):
                    led = ledger[:, b0:b0 + gn, i * H:(i + 1) * H]
                    st = ep.tile([128, 4, 6], F32, tag="nst", bufs=2)
                    mv = ep.tile([128, 4, 2], F32, tag="nmv", bufs=2)
                    for b in range(gn):
                        nc.vector.bn_stats(out=st[:, b, :], in_=led[:, b, :])
                        nc.vector.bn_aggr(out=mv[:, b, :], in_=st[:, b, :])
                    # rstd = exp(-0.5*ln(var+eps))
                    rstd = ep.tile([128, 4], F32, tag="nrstd", bufs=2)
                    nc.scalar.activation(
                        out=rstd[:, 0:gn], in_=mv[:, 0:gn, 1], func=AF.Ln,
                        bias=epsb[:], scale=1.0)
                    nc.scalar.activation(
                        out=rstd[:, 0:gn], in_=rstd[:, 0:gn], func=AF.Exp,
                        bias=0.0, scale=-0.5)
                    nmr = ep.tile([128, 4], F32, tag="nnmr", bufs=2)
                    nc.vector.tensor_tensor(
                        out=nmr[:, 0:gn], in0=mv[:, 0:gn, 0], in1=rstd[:, 0:gn],
                        op=ALU.mult)
                    u2 = ep.tile([128, 4, H], F32, tag="nu2", bufs=2)
                    for b in range(gn):
                        nc.vector.tensor_scalar(
                            out=u2[:, b, :], in0=led[:, b, :],
                            scalar1=rstd[:, b:b + 1], scalar2=nmr[:, b:b + 1],
                            op0=ALU.mult, op1=ALU.subtract)
                    u2g = u2[:, 0:gn, :]
                    bc = [128, gn, H]
                    if not f["ln_unit"]:
                        nc.vector.tensor_tensor(
                            out=u2g, in0=u2g,
                            in1=gbb[i][:].unsqueeze(1).broadcast_to(bc), op=ALU.mult)
                        nc.vector.tensor_tensor(
                            out=u2g, in0=u2g,
                            in1=bbb[i][:].unsqueeze(1).broadcast_to(bc), op=ALU.add)
                    r4 = ep.tile([128, 4, H], F32, tag="nr4", bufs=2)
                    nc.vector.tensor_scalar(
                        out=r4[:, 0:gn, :], in0=u2g, scalar1=0.0, scalar2=None,
                        op0=ALU.max)
                    m4 = ep.tile([128, 4, H], F32, tag="nm4", bufs=2)
                    if f["alpha_const"][i] is not None:
                        nc.vector.tensor_scalar(
                            out=m4[:, 0:gn, :], in0=u2g, scalar1=0.0,
                            scalar2=f["alpha_const"][i], op0=ALU.min, op1=ALU.mult)
                    else:
                        nc.vector.tensor_scalar(
                            out=m4[:, 0:gn, :], in0=u2g, scalar1=0.0, scalar2=None,
                            op0=ALU.min)
                        nc.vector.tensor_tensor(
                            out=m4[:, 0:gn, :], in0=m4[:, 0:gn, :],
                            in1=abb[i][:].unsqueeze(1).broadcast_to(bc), op=ALU.mult)
                    nc.vector.tensor_tensor(
                        out=usc[:, b0:b0 + gn, :], in0=r4[:, 0:gn, :],
                        in1=m4[:, 0:gn, :], op=ALU.add)
                    A4 = ep.tile([128, 4, H], F32, tag="nA4", bufs=2)
                    nc.scalar.activation(
                        out=A4[:, 0:gn, :], in_=r4[:, 0:gn, :], func=AF.Exp,
                        bias=0.0, scale=tg)
                    ab0 = ep.tile([128, 4, H], tdt, tag="nab0", bufs=2)
                    nc.scalar.activation(
                        out=ab0[:, 0:gn, :], in_=A4[:, 0:gn, :], func=AF.Copy,
                        bias=-1.0, scale=1.0)
                    r8 = ep.tile([128, 4, H], F32, tag="nr8", bufs=2)
                    nc.vector.tensor_scalar(
                        out=r8[:, 0:gn, :], in0=r4[:, 0:gn, :], scalar1=0.125,
                        scalar2=None, op0=ALU.mult)
                    ab1 = ep.tile([128, 4, H], tdt, tag="nab1", bufs=2)
                    nc.vector.tensor_tensor(
                        out=ab1[:, 0:gn, :], in0=r8[:, 0:gn, :],
                        in1=A4[:, 0:gn, :], op=ALU.mult)
                    nc.sync.dma_start(
                        out=tpbc[:, b0:b0 + gn, 0:H], in_=ab0[:, 0:gn, :])
                    nc.sync.dma_start(
                        out=tpbc[:, b0:b0 + gn, H:2 * H], in_=ab1[:, 0:gn, :])

                return node_group

            # ---------------- edge phase machinery -------------------------
            def edge_phase(tag, mode, tableA, tableB, elem, estep, gdt,
                           drain_fn, group_fn, big_calls=True):
                """big_calls: one 6144-idx gather per 16 bins (2 sel windows);
                else one 3072-idx gather per 8-bin window."""
                NW = 2 if big_calls else 1
                with (
                    tc.tile_pool(name=f"ep_{tag}", bufs=1) as ep,
                    tc.tile_pool(name=f"epp_{tag}", bufs=4, space="PSUM") as epp,
                    tc.tile_pool(name=f"mpp_{tag}", bufs=3, space="PSUM") as mpp,
                ):
                    ga = gb = None
                    for sc in range(NSC):
                        selb = ep.tile([128, 48, 32], BF16, tag="selb", bufs=3)
                        nc.vector.tensor_tensor(
                            out=selb[:],
                            in0=dall[:, sc * 48:(sc + 1) * 48]
                            .unsqueeze(2).broadcast_to([128, 48, 32]),
                            in1=iota[:].unsqueeze(1).broadcast_to([128, 48, 32]),
                            op=ALU.is_equal)
                        if sc % NW == 0:
                            vA = s.validA16[sc // 2] if big_calls else s.validA8[sc]
                            vB = s.validB16[sc // 2] if big_calls else s.validB8[sc]
                            ga = ep.tile([128, 24 * NW, elem], gdt, tag="ga",
                                         bufs=3 if not big_calls else 2)
                            gb = ep.tile([128, 24 * NW, elem], gdt, tag="gb",
                                         bufs=3 if not big_calls else 2)
                            if vA > 0:
                                raw_dma_gather(
                                    nc.gpsimd, ga[:], tableA,
                                    idxA_sb[:, sc * (WIN // 16):
                                            (sc + NW) * (WIN // 16)],
                                    WIN * NW, reg_of(vA), elem, estep,
                                    queue_num=0)
                            if vB > 0:
                                raw_dma_gather(
                                    nc.gpsimd, gb[:], tableB,
                                    idxB_sb[:, sc * (WIN // 16):
                                            (sc + NW) * (WIN // 16)],
                                    WIN * NW, reg_of(vB), elem, estep,
                                    queue_num=1)
                        cb = (sc % NW) * 24
                        for lb in range(2):
                            blk = sc * 2 + lb
                            if mode == "conv":
                                ps = epp.tile([32, 128], F32, tag="eps",
                                              space="PSUM")
                            else:
                                ps = epp.tile([128, 128], F32, tag="eps",
                                              space="PSUM")
                            for j in range(4):
                                binl = lb * 4 + j
                                gbin = sc * SC_BINS + binl
                                ta = int(s.tmaxA[gbin])
                                tb = int(s.tmaxB[gbin])
                                if ta + tb == 0:
                                    ta = 1  # zero the psum region via zero sel
                                for half, gt, tn in ((0, ga, ta), (1, gb, tb)):
                                    for t in range(tn):
                                        col = cb + binl * TPB + t
                                        scol = half * 24 + binl * TPB + t
                                        first = half == 0 and t == 0 if ta > 0                                             else (half == 1 and t == 0)
                                        last = (half == 1 and t == tb - 1) if tb > 0                                             else (half == 0 and t == ta - 1)
                                        if mode == "conv":
                                            nc.tensor.matmul(
                                                out=ps[:, 32 * j:32 * j + 32],
                                                lhsT=gt[:, col, :],
                                                rhs=selb[:, scol, :],
                                                start=first, stop=last)
                                        else:
                                            nc.tensor.matmul(
                                                out=ps[32 * j:32 * j + 32, :],
                                                lhsT=selb[:, scol, :],
                                                rhs=gt[:, col, :],
                                                start=first, stop=last,
                                                tile_position=(0, 32 * j))
                            drain_fn(blk, ps, ep, mpp)
                            if group_fn is not None:
                                if blk >= NBLK - 4:
                                    group_fn(blk, 1, ep)
                                elif blk % 4 == 3:
                                    group_fn(blk - 3, 4, ep)

            # ---------------- conv ----------------
            def conv_drain(blk, ps, ep, mpp):
                xs5 = ep.tile([F_IN, 128], BF16, tag="cxs5", bufs=3)
                nc.vector.tensor_tensor(
                    out=xs5[:], in0=ps[0:F_IN, :],
                    in1=xdT[:, blk * 128:(blk + 1) * 128], op=ALU.add)
                h0 = mpp.tile([128, H], F32, tag="mlp", space="PSUM")
                nc.tensor.matmul(out=h0[:], lhsT=xs5[:], rhs=Wc5[:],
                                 start=True, stop=True)
                if btotb is None:
                    nc.scalar.activation(
                        out=ledger[:, blk, 0:H], in_=h0[:], func=AF.Relu,
                        bias=0.0, scale=dinv[:, blk:blk + 1])
                else:
                    t1 = ep.tile([128, H], F32, tag="ct1", bufs=3)
                    nc.vector.tensor_scalar(
                        out=t1[:], in0=h0[:], scalar1=dinv[:, blk:blk + 1],
                        scalar2=None, op0=ALU.mult)
                    nc.vector.tensor_tensor(out=t1[:], in0=t1[:], in1=btotb[:],
                                            op=ALU.add)
                    nc.vector.tensor_scalar(
                        out=ledger[:, blk, 0:H], in0=t1[:], scalar1=0.0,
                        scalar2=None, op0=ALU.max)

            edge_phase("cv", "conv",
                       ctab_d[0:s.SPLIT, 0:32], ctab_d[s.SPLIT:2 * s.SPLIT, 0:32],
                       32, 128, BF16, conv_drain, make_node_group(0))
            allgather(tab0, agg0)

            # ---------------- GEN layers ----------------
            def make_gen_drain(i):
                def gen_drain(blk, ps, ep, mpp):
                    lnv = ep.tile([128, H], F32, tag="glnv", bufs=3)
                    nc.scalar.activation(
                        out=lnv[:], in_=ps[:, 0:H], func=AF.Ln,
                        bias=indeg8[:, blk:blk + 1], scale=0.125)
                    sden = ep.tile([128, H], F32, tag="gsden", bufs=3)
                    nc.scalar.activation(
                        out=sden[:], in_=lnv[:], func=AF.Exp, bias=0.0, scale=-1.0)
                    agg = ep.tile([128, H], F32, tag="gagg", bufs=3)
                    nc.vector.tensor_tensor(
                        out=agg[:], in0=ps[:, H:2 * H], in1=sden[:], op=ALU.mult)
                    nc.vector.tensor_tensor(
                        out=agg[:], in0=agg[:], in1=usc[:, blk, :], op=ALU.add)
                    tps = mpp.tile([H, 128], F32, tag="mlp", space="PSUM")
                    nc.tensor.transpose(out=tps[:], in_=agg[:], identity=ident[:])
                    aggT = ep.tile([H, 128], BF16, tag="gaggT", bufs=3)
                    nc.scalar.activation(
                        out=aggT[:], in_=tps[:], func=AF.Copy, bias=0.0, scale=1.0)
                    z1ps = mpp.tile([128, 128], F32, tag="mlp", space="PSUM")
                    nc.tensor.matmul(out=z1ps[:], lhsT=W1[i][:], rhs=aggT[:],
                                     start=True, stop=True)
                    z1r = ep.tile([128, 128], BF16, tag="gz1r", bufs=3)
                    if b1[i] is None:
                        nc.scalar.activation(
                            out=z1r[:], in_=z1ps[:], func=AF.Relu, bias=0.0,
                            scale=1.0)
                    else:
                        nc.scalar.activation(
                            out=z1r[:], in_=z1ps[:], func=AF.Relu, bias=b1[i][:],
                            scale=1.0)
                    z2ps = mpp.tile([128, H], F32, tag="mlp", space="PSUM")
                    nc.tensor.matmul(out=z2ps[:], lhsT=z1r[:], rhs=W2[i][:],
                                     start=True, stop=True)
                    z2sb = ep.tile([128, H], BF16, tag="gz2sb", bufs=3)
                    nc.scalar.activation(
                        out=z2sb[:], in_=z2ps[:], func=AF.Copy, bias=0.0, scale=1.0)
                    if b2b[i] is not None:
                        nc.vector.tensor_tensor(
                            out=z2sb[:], in0=z2sb[:], in1=b2b[i][:], op=ALU.add)
                    nc.vector.tensor_tensor(
                        out=ledger[:, blk, (i + 1) * H:(i + 2) * H],
                        in0=z2sb[:], in1=ledger[:, blk, i * H:(i + 1) * H],
                        op=ALU.add)
                    if i == L - 1 and blk == 0:
                        # slot 0 = pooling max-pad token
                        nc.vector.memset(ledger[0:1, 0, 0:CH], -3.0e38)

                return gen_drain

            for i in range(L):
                if i == 0:
                    tA = agg0[0:s.SPLIT, :]
                    tB = agg0[s.SPLIT:2 * s.SPLIT, :]
                    elem, estep, gdt = 128, 128, BF16
                else:
                    tA = agg8[0:s.SPLIT, 0:128]
                    tB = agg8[s.SPLIT:2 * s.SPLIT, 0:128]
                    elem, estep, gdt = 128, T8W, T8DT
                edge_phase(f"g{i}", "gen", tA, tB, elem, estep, gdt,
                           make_gen_drain(i),
                           make_node_group(i + 1) if i < L - 1 else None,
                           big_calls=(i != 0))
                if i < L - 1:
                    allgather(tab8, agg8)

            # ---------------- pooling + head ----------------
            with (
                tc.tile_pool(name="pool", bufs=1) as qp,
                tc.tile_pool(name="poolps", bufs=1, space="PSUM") as qpp,
            ):
                # mean pool: gsel matmul over ledger blocks
                mps = qpp.tile([GPC, CH], F32, tag="mps", space="PSUM")
                for blk in range(NBLK):
                    nc.tensor.matmul(
                        out=mps[:],
                        lhsT=gselt[:, blk * GPC:(blk + 1) * GPC],
                        rhs=ledger[:, blk, :],
                        start=(blk == 0), stop=(blk == NBLK - 1))
                xmean = qp.tile([GPC, CH], F32, tag="xmean")
                nc.vector.tensor_scalar(
                    out=xmean[:], in0=mps[:], scalar1=pinv[:, 0:1], scalar2=None,
                    op0=ALU.mult)
                pooled = qp.tile([128, 4, GPC], F32)
                for k in range(2):
                    tp = qpp.tile([128, GPC], F32, tag=f"tp{k}", space="PSUM")
                    nc.tensor.transpose(
                        out=tp[:], in_=xmean[:, k * 128:(k + 1) * 128],
                        identity=ident[0:GPC, 0:GPC])
                    nc.scalar.activation(
                        out=pooled[:, k, :], in_=tp[:], func=AF.Copy, bias=0.0,
                        scale=1.0)
                # max pool: SBUF transpose-gathers (per 2 graphs)
                gnidx_reg = nc.gpsimd.to_reg(2 * SG)
                ledflat = ledger[:].rearrange("p b c -> p (b c)")
                PGS = 2 * SG
                for k in range(GPC // 2):
                    grid = qp.tile([128, 2, PGS], BF16, tag="grid", bufs=3)
                    nc.gpsimd.dma_gather(
                        grid[:, :, :], ledflat,
                        gix[:, k * (PGS // 16):(k + 1) * (PGS // 16)],
                        PGS, gnidx_reg, CH,
                        transpose=True,
                        sbuf_tokens_per_rank=128,
                        sbuf_free_dim_per_rank=CH * 2,
                        queue_num=k % NQUEUES)
                    for half in range(2):
                        red = qp.tile([128, 2], F32, tag="red", bufs=3)
                        nc.vector.reduce_max(
                            out=red[:],
                            in_=grid[:, half, :].rearrange("p (m t) -> p m t", t=SG),
                            axis=mybir.AxisListType.X)
                        nc.vector.tensor_tensor(
                            out=pooled[:, 2 + half, 2 * k:2 * k + 2],
                            in0=red[:], in1=pmaxt[:, 2 * k:2 * k + 2], op=ALU.mult)
                nc.sync.dma_start(
                    out=pool_in.ap().rearrange("k p g -> p k g"), in_=pooled[:])
                if MOCK_COLLECTIVES:
                    nc.sync.dma_start(out=pool_out[0, :, :, :], in_=pool_in[:, :, :])
                else:
                    nc.gpsimd.collective_compute(
                        "AllGather", ALU.bypass, replica_groups=RG,
                        ins=[pool_in[:, :, :]], outs=[pool_out[:, :, :, :]])
                # head (f32)
                hps = qpp.tile([128, s.G], F32, tag="hps", space="PSUM")
                pk = []
                for k in range(4):
                    t = qp.tile([128, NCORES, GPC], F32, tag=f"pk{k}")
                    nc.sync.dma_start(
                        out=t[:], in_=pool_out[:, k, :, :].rearrange("r p g -> p r g"))
                    pk.append(t)
                for k in range(4):
                    nc.tensor.matmul(
                        out=hps[:], lhsT=l1W[k][:],
                        rhs=pk[k][:].rearrange("p r g -> p (r g)"),
                        start=(k == 0), stop=(k == 3))
                hz1 = qp.tile([128, s.G], F32)
                nc.scalar.activation(
                    out=hz1[:], in_=hps[:], func=AF.Relu, bias=l1b[:], scale=1.0)
                h2ps = qpp.tile([H, s.G], F32, tag="h2ps", space="PSUM")
                nc.tensor.matmul(out=h2ps[:], lhsT=l2W[:], rhs=hz1[:],
                                 start=True, stop=True)
                hz2 = qp.tile([H, s.G], F32)
                nc.scalar.activation(
                    out=hz2[:], in_=h2ps[:], func=AF.Relu, bias=l2b[:], scale=1.0)
                ops = qpp.tile([1, s.G], F32, tag="ops", space="PSUM")
                nc.tensor.matmul(out=ops[:], lhsT=oW[:], rhs=hz2[:],
                                 start=True, stop=True)
                osb = qp.tile([1, s.G], F32)
                nc.vector.tensor_scalar(
                    out=osb[:], in0=ops[:], scalar1=float(f["out_b"][0]),
                    scalar2=None, op0=ALU.add)
                nc.sync.dma_start(out=out_d.ap().rearrange("g one -> one g"),
                                  in_=osb[:])

    nc.compile()
    return nc


# ---------------------------------------------------------------- entry
def kernel(**inputs) -> np.ndarray:
    x = np.asarray(inputs["x"], np.float32)
    ei = np.asarray(inputs["edge_index"], np.int64)
    bi = np.asarray(inputs["batch_idx"], np.int64)
    G = 256
    s = build_schedule(ei, bi, G)
    f = fold_weights(inputs)
    maps = build_inmaps(s, x, f)
    nc = build_nc(s, f)
    res = run_bass_kernel_spmd(nc, maps, core_ids=list(range(NCORES)))
    return np.asarray(res.results[0]["out"], np.float32)
